# revision 4
# baseline (speedup 1.0000x reference)
"""GCN (5-layer) + global mean pool + MLP head on 8 trn2 NeuronCores.

Transposed ap_gather design, instruction-count-minimized:
  - Factorize GCN norm: with dis = rsqrt(deg), the table carries
    t_0 = dis*(x@W1) and t_l = dis*h_l (l>=1). A layer is then
      h_l[v] = relu(dis[v] * (W_l^T @ sum_{u->v + self} t_{l-1}[u]) + b_l)
    where W_1 = I (already applied in t_0). The W product rides the fold
    matmul's lhsT (replicated W), so no separate W matmuls are needed, and
    self-loops are plain gather slots.
  - Full t table in SBUF transposed: tab[16g+h, j] = t[g*12500+j][h].
    Per layer, each gpsimd core ap_gathers its source-group's in-edge
    messages slot-major (per-group degree-sorted dests, shared pow2 K-runs),
    DVE halving-reduces the slots, one small ap_gather realigns each group
    to canonical dest order, and PE matmuls with replicated-W lhsT fold the
    8 group partials into pre-activations [16, D].
  - Epilogue (dis*, +b, relu, dis*) runs on 2048-dest chunks; t chunks DMA
    to DRAM and one f32 AllGather + one DMA rebuilds the next table.
  - Mean-pool reuses the same gather/reduce/align/fold machinery.
"""
import numpy as np

import concourse.bass as bass
import concourse.bacc as bacc
import concourse.tile as tile
import concourse.mybir as mybir
from concourse.bass2jax import run_bass_via_pjrt

F32 = mybir.dt.float32
F32R = mybir.dt.float32r
I16 = mybir.dt.int16
AL = mybir.AluOpType

P = 128
N_NODES = 100000
N_GRAPHS = 1000
HID = 16
C = 8
NPC = N_NODES // C          # 12500
GPC = N_GRAPHS // C         # 125
D_PAD = 12800               # padded dests per core
MMW = 512                   # matmul out width (one PSUM bank)
CHW = 4096                  # epilogue dest-chunk width (8 banks)
TBLW = D_PAD                # table cols
ZERO_COL = NPC              # tab col guaranteed zero (pad dest)
SLAB_W = NPC + 16           # slab cols (+16 zero cols)
PR_PAD = 128                # padded graph count per core
PRW = GPC + 16              # pool_raw width
GCHUNK = 8192
ABLK = 4096                 # must be a multiple of CHW
RRELU_SLOPE = (1.0 / 8.0 + 1.0 / 3.0) / 2.0


def _echunks_of(j):
    """Epilogue chunks: (col0, width, psum partition base, disT col base)."""
    out = []
    col = 0
    j2 = 0
    while col < D_PAD:
        w = min(CHW, D_PAD - col)
        out.append((col, w, 32 * (j2 % 4), CHW * (j2 // 4)))
        col += w
        j2 += 1
    return out


EPI_CHUNKS = _echunks_of(0)
DISW = 4096


# ---------------------------------------------------------------------------
# host preprocessing
# ---------------------------------------------------------------------------

def _slot_plan(counts, gchunk):
    """counts: [C, C, R] per-(core c, group g) item counts. Builds shared
    pow2-quantized slot structure + per-(c,g) orderings (see kernel_new
    docstring). Returns rank[c,g,item], chunks, per-rank colmaps, S_tot."""
    Ccores, Cg, R = counts.shape
    order = np.argsort(-counts, axis=2, kind="stable")
    rank = np.empty_like(order)
    ar = np.arange(R)
    for c in range(Ccores):
        for g in range(Cg):
            rank[c, g, order[c, g]] = ar
    dsort = np.take_along_axis(counts, order, 2)
    K_sh = dsort.max(axis=(0, 1))
    K_sh = np.maximum(K_sh, 1)
    # quantize up to powers of two (fewer runs -> fewer reduce instructions)
    K_sh = (2 ** np.ceil(np.log2(K_sh))).astype(np.int64)
    assert (np.diff(K_sh) <= 0).all()

    runs = []
    r = 0
    while r < R:
        K = int(K_sh[r])
        r1 = int(np.searchsorted(-K_sh, -K, side="right"))
        maxC = max(1, gchunk // K)
        while r < r1:
            Cr = min(maxC, r1 - r)
            runs.append((r, Cr, K))
            r += Cr

    run_col = np.zeros(R, np.int64)
    run_C = np.ones(R, np.int64)
    run_off = np.zeros(R, np.int64)
    chunks = []
    slot_base = 0
    cur = []
    cur_slots = 0

    def flush_tree():
        nonlocal cur, cur_slots, slot_base
        if not cur:
            return
        n_idx = -(-cur_slots // 16) * 16
        chunks.append(("tree", n_idx, slot_base, list(cur)))
        slot_base += n_idx
        cur = []
        cur_slots = 0

    for (r0, Cr, K) in runs:
        if K >= 2:
            if cur_slots + Cr * K > gchunk:
                flush_tree()
            cur.append((cur_slots, r0, Cr, K))
            cur_slots += Cr * K
        else:
            flush_tree()
            rr = r0
            while rr < r0 + Cr:
                n = min(gchunk, r0 + Cr - rr)
                n_idx = -(-n // 16) * 16
                chunks.append(("direct", n_idx, slot_base, rr))
                run_col[rr:rr + n] = slot_base + np.arange(n)
                slot_base += n_idx
                rr += n
    flush_tree()

    for kind, n_idx, sbase, payload in chunks:
        if kind == "tree":
            for (tile_base, r0, Cr, K) in payload:
                run_col[r0:r0 + Cr] = sbase + tile_base
                run_C[r0:r0 + Cr] = Cr
                run_off[r0:r0 + Cr] = np.arange(Cr)
    S_tot = slot_base
    assert S_tot % 16 == 0
    return dict(rank=rank, chunks=chunks, run_col=run_col, run_C=run_C,
                run_off=run_off, S_tot=S_tot)


def _wrap16(stream):
    """[8 groups, S] -> [128, S//16]: element i of group g at
    (16g + i%16, i//16)."""
    Cg, S = stream.shape
    assert Cg == 8 and S % 16 == 0
    return stream.reshape(8, S // 16, 16).transpose(0, 2, 1).reshape(
        128, S // 16).astype(np.int16)


def _slots_from(plan_part, keys, ranks, vals, R_per, n_streams, S_tot):
    """Scatter vals into idx streams [n_streams, S_tot] by slot structure."""
    order = np.lexsort((keys,))
    ks = keys[order]
    starts = np.concatenate([[True], ks[1:] != ks[:-1]])
    first = np.flatnonzero(starts)
    gid = np.cumsum(starts) - 1
    slot = np.arange(len(keys)) - first[gid]
    rs = ranks[order]
    col = plan_part["run_col"][rs] + slot * plan_part["run_C"][rs] \
        + plan_part["run_off"][rs]
    stream = np.full((n_streams, S_tot), ZERO_COL, dtype=np.int64)
    sid = keys[order] // R_per
    stream.reshape(-1)[sid * S_tot + col] = vals[order]
    return stream


def _preprocess(x, edge_index, batch):
    x = np.asarray(x, dtype=np.float32)
    src0 = np.asarray(edge_index[0], dtype=np.int64)
    dst0 = np.asarray(edge_index[1], dtype=np.int64)
    batch = np.asarray(batch, dtype=np.int64)

    deg = np.bincount(dst0, minlength=N_NODES).astype(np.float32) + 1.0
    dis = 1.0 / np.sqrt(deg)

    # append self loops as regular edges
    loop = np.arange(N_NODES, dtype=np.int64)
    src = np.concatenate([src0, loop])
    dst = np.concatenate([dst0, loop])
    E = src.shape[0]

    c_of = dst // NPC
    g_of = src // NPC

    key_cgv = (c_of * C + g_of) * NPC + (dst % NPC)
    cnt = np.bincount(key_cgv, minlength=C * C * NPC).reshape(C, C, NPC)
    ep = _slot_plan(cnt, GCHUNK)
    rank_e = ep["rank"][c_of, g_of, dst % NPC]
    idx_stream = _slots_from(ep, (c_of * C + g_of) * NPC + rank_e, rank_e,
                             src % NPC, NPC, C * C, ep["S_tot"])
    idx_stream = idx_stream.reshape(C, C, ep["S_tot"])

    align = np.full((C, C, D_PAD), NPC, dtype=np.int64)
    align[:, :, :NPC] = ep["rank"]

    # pooling
    c_p = batch // GPC
    g_n = np.arange(N_NODES) // NPC
    gamma = batch % GPC
    cntp = np.bincount((c_p * C + g_n) * GPC + gamma,
                       minlength=C * C * GPC).reshape(C, C, GPC)
    pp = _slot_plan(cntp, GCHUNK)
    rank_n = pp["rank"][c_p, g_n, gamma]
    pidx_stream = _slots_from(pp, (c_p * C + g_n) * GPC + rank_n, rank_n,
                              loop % NPC, GPC, C * C, pp["S_tot"])
    pidx_stream = pidx_stream.reshape(C, C, pp["S_tot"])

    palign = np.full((C, C, PR_PAD), GPC, dtype=np.int64)
    palign[:, :, :GPC] = pp["rank"]

    cnt_graph = np.maximum(np.bincount(batch, minlength=N_GRAPHS), 1.0)
    rcp = (1.0 / cnt_graph).astype(np.float32)

    per_core = []
    for c in range(C):
        dis_pad = np.zeros(D_PAD, np.float32)
        dis_pad[:NPC] = dis[c * NPC:(c + 1) * NPC]
        disT = np.zeros((P, DISW), np.float32)
        for (col0, w, pb, cb) in EPI_CHUNKS:
            disT[pb:pb + 16, cb:cb + w] = dis_pad[col0:col0 + w]

        rcpc = np.zeros((16, PR_PAD), np.float32)
        rcpc[:, :GPC] = rcp[c * GPC:(c + 1) * GPC]

        per_core.append(dict(
            gidx=_wrap16(idx_stream[c]), aidx=_wrap16(align[c]),
            pidx=_wrap16(pidx_stream[c]), paidx=_wrap16(palign[c]),
            disT=disT, rcpcnt=rcpc))

    plan = dict(echunks=ep["chunks"], S_tot=ep["S_tot"],
                pchunks=pp["chunks"], Sp_tot=pp["S_tot"], dis=dis)
    return plan, per_core


def _make_tab1(x, W1, dis):
    y1 = dis[:, None] * (np.asarray(x, np.float32) @ np.asarray(W1, np.float32))
    tab1 = np.zeros((P, TBLW), np.float32)
    for g in range(C):
        tab1[16 * g:16 * g + 16, :NPC] = y1[g * NPC:(g + 1) * NPC].T
    return tab1


# ---------------------------------------------------------------------------
# program
# ---------------------------------------------------------------------------

def _build_program(plan, reps=1, mode="full"):
    echunks = plan["echunks"]
    S_tot = plan["S_tot"]
    pchunks = plan["pchunks"]
    Sp_tot = plan["Sp_tot"]

    nc = bacc.Bacc(None, target_bir_lowering=False, num_devices=C)

    tab1_in = nc.dram_tensor("tab1", [P, TBLW], F32, kind="ExternalInput")
    gidx_in = nc.dram_tensor("gidx", [P, S_tot // 16], I16, kind="ExternalInput")
    aidx_in = nc.dram_tensor("aidx", [P, D_PAD // 16], I16, kind="ExternalInput")
    pidx_in = nc.dram_tensor("pidx", [P, Sp_tot // 16], I16, kind="ExternalInput")
    paidx_in = nc.dram_tensor("paidx", [P, PR_PAD // 16], I16, kind="ExternalInput")
    disT_in = nc.dram_tensor("disT", [P, DISW], F32, kind="ExternalInput")
    repI_in = nc.dram_tensor("repI", [P, 16], F32, kind="ExternalInput")
    repW_in = nc.dram_tensor("repW", [P, 64], F32, kind="ExternalInput")
    rcp_in = nc.dram_tensor("rcpcnt", [16, PR_PAD], F32, kind="ExternalInput")
    b_in = nc.dram_tensor("bs", [P, 5], F32, kind="ExternalInput")
    l1w_in = nc.dram_tensor("lin1_w", [HID, HID], F32, kind="ExternalInput")
    l1b_in = nc.dram_tensor("lin1_b", [HID, 1], F32, kind="ExternalInput")
    l2w_in = nc.dram_tensor("lin2_w", [HID, 1], F32, kind="ExternalInput")
    l2b_in = nc.dram_tensor("lin2_b", [1, 1], F32, kind="ExternalInput")
    out_t = nc.dram_tensor("out", [1, PR_PAD], F32, kind="ExternalOutput")

    ag_in = nc.dram_tensor("ag_in", [HID, D_PAD], F32)
    ag_out = nc.dram_tensor("ag_out", [P, D_PAD], F32, addr_space="Shared")

    _ = nc.partition_id_tensor  # SPMD marker

    def ap3(t_ap, n):
        return bass.AP(t_ap.tensor, t_ap.offset,
                       [[t_ap.ap[0][0], t_ap.ap[0][1]], [1, n], [1, 1]])

    with tile.TileContext(nc) as tc:
        import contextlib
        with contextlib.ExitStack() as ctx:
            sbp = ctx.enter_context(tc.tile_pool(name="persist", bufs=1))
            gp = ctx.enter_context(tc.tile_pool(name="g", bufs=1))
            alp = ctx.enter_context(tc.tile_pool(name="al", bufs=1))
            hp = ctx.enter_context(tc.tile_pool(name="h", bufs=1))
            hp2 = ctx.enter_context(tc.tile_pool(name="h2", bufs=3))
            psf = ctx.enter_context(tc.tile_pool(name="psf", bufs=1, space="PSUM"))

            tab = sbp.tile([P, TBLW], F32, name="tab")
            slab = sbp.tile([P, SLAB_W], F32, name="slab")
            pool_raw = sbp.tile([P, PRW], F32, name="pool_raw")
            gidx = sbp.tile([P, S_tot // 16], I16, name="gidx")
            aidx = sbp.tile([P, D_PAD // 16], I16, name="aidx")
            pidx = sbp.tile([P, Sp_tot // 16], I16, name="pidx")
            paidx = sbp.tile([P, PR_PAD // 16], I16, name="paidx")
            disT = sbp.tile([P, DISW], F32, name="disT")
            repI = sbp.tile([P, 16], F32, name="repI")
            repW = sbp.tile([P, 64], F32, name="repW")
            rcpc = sbp.tile([16, PR_PAD], F32, name="rcpc")
            b_sb = sbp.tile([P, 5], F32, name="bs")
            l1w = sbp.tile([HID, HID], F32, name="l1w")
            l1b = sbp.tile([HID, 1], F32, name="l1b")
            l2w = sbp.tile([HID, 1], F32, name="l2w")
            l2b = sbp.tile([1, 1], F32, name="l2b")

            for sb, dr in [(gidx, gidx_in), (aidx, aidx_in), (pidx, pidx_in),
                           (paidx, paidx_in), (disT, disT_in), (repI, repI_in),
                           (repW, repW_in), (rcpc, rcp_in), (b_sb, b_in),
                           (l1w, l1w_in), (l1b, l1b_in), (l2w, l2w_in),
                           (l2b, l2b_in)]:
                nc.sync.dma_start(sb[:], dr[:])

            def gather(idx_tile, col0, n_idx, out_ap, in_tile, n_elems):
                nc.gpsimd.ap_gather(
                    out_ap=out_ap,
                    in_ap=ap3(in_tile[:], n_elems),
                    idxs_ap=idx_tile[:, col0:col0 + n_idx // 16],
                    channels=P, num_elems=n_elems, d=1, num_idxs=n_idx)

            tog = [0]

            def gather_reduce(chunks, idx_tile, src_tile, src_w, dst_tile):
                for kind, n_idx, sbase, payload in chunks:
                    if kind == "direct":
                        r0 = payload
                        out = bass.AP(dst_tile[:].tensor,
                                      dst_tile[:].offset + r0,
                                      [[dst_tile[:].ap[0][0], P],
                                       [1, n_idx], [1, 1]])
                        gather(idx_tile, sbase // 16, n_idx, out,
                               src_tile, src_w)
                        continue
                    g = gp.tile([P, GCHUNK + 16], F32, tag="g", name="g")
                    gather(idx_tile, sbase // 16, n_idx, ap3(g[:], n_idx),
                           src_tile, src_w)
                    for (tb, r0, Cr, K) in payload:
                        k = K
                        while k > 2:
                            h = k // 2
                            eng = nc.vector if tog[0] % 2 == 0 else nc.gpsimd
                            tog[0] += 1
                            eng.tensor_add(
                                out=g[:, tb:tb + h * Cr],
                                in0=g[:, tb:tb + h * Cr],
                                in1=g[:, tb + h * Cr:tb + 2 * h * Cr])
                            k = h
                        eng = nc.vector if tog[0] % 2 == 0 else nc.gpsimd
                        tog[0] += 1
                        eng.tensor_add(
                            out=dst_tile[:, r0:r0 + Cr],
                            in0=g[:, tb:tb + Cr],
                            in1=g[:, tb + Cr:tb + 2 * Cr])

            def layer(l, do_gather=True):
                if do_gather:
                    gather_reduce(echunks, gidx, tab, TBLW, slab)
                lw = repI[:] if l == 1 else repW[:, 16 * (l - 2):16 * (l - 1)]
                ht = hp.tile([P, CHW], F32, tag="h", name="ht")
                for a0 in range(0, D_PAD, ABLK):
                    aw = min(ABLK, D_PAD - a0)
                    al = alp.tile([P, ABLK], F32, tag="al", name="al")
                    gather(aidx, a0 // 16, aw, ap3(al[:], aw), slab, SLAB_W)
                    for (col0, w, pb, cb) in EPI_CHUNKS:
                        if col0 < a0 or col0 >= a0 + aw:
                            continue
                        pm = psf.tile([HID, CHW], F32, tag="f", space="PSUM",
                                      name="pm")
                        for m0 in range(0, w, MMW):
                            mw = min(MMW, w - m0)
                            nc.tensor.matmul(
                                out=pm[:, m0:m0 + mw],
                                lhsT=lw,
                                rhs=al[:, col0 - a0 + m0:col0 - a0 + m0 + mw
                                       ],
                                start=True, stop=True)
                        htv = ht[pb:pb + 16, :w]
                        nc.vector.tensor_mul(out=htv, in0=pm[:, :w],
                                             in1=disT[pb:pb + 16, cb:cb + w])
                        nc.vector.tensor_scalar(
                            out=htv, in0=htv,
                            scalar1=b_sb[pb:pb + 16, l - 1:l],
                            scalar2=0.0, op0=AL.add, op1=AL.max)
                        if l < 5:
                            nc.vector.tensor_mul(
                                out=htv, in0=htv,
                                in1=disT[pb:pb + 16, cb:cb + w])
                        dst = bass.AP(ag_in[:].tensor, col0,
                                      [[D_PAD, HID], [1, w]])
                        nc.sync.dma_start(dst, htv)

            def allgather():
                nc.gpsimd.collective_compute(
                    "AllGather", AL.bypass,
                    replica_groups=[list(range(C))],
                    ins=[ag_in[:]], outs=[ag_out[:]])
                nc.sync.dma_start(tab[:], ag_out[:])

            def pooling_head():
                gather_reduce(pchunks, pidx, tab, TBLW, pool_raw)
                alp_t = alp.tile([P, ABLK], F32, tag="al", name="alp")
                gather(paidx, 0, PR_PAD, ap3(alp_t[:], PR_PAD),
                       pool_raw, PRW)
                pm = psf.tile([HID, CHW], F32, tag="f", space="PSUM",
                              name="pmp")
                nc.tensor.matmul(out=pm[:, :PR_PAD], lhsT=repI[:],
                                 rhs=alp_t[:, :PR_PAD],
                                 start=True, stop=True)
                pooled = hp2.tile([HID, PR_PAD], F32, tag="pool", name="pooled")
                nc.vector.tensor_mul(out=pooled[:, :PR_PAD],
                                     in0=pm[:, :PR_PAD], in1=rcpc[:])

                def rrelu_block(pm_ap, b_ap, rows, out_tile, tmp_tile):
                    nc.vector.tensor_scalar(out=tmp_tile[:rows, :PR_PAD],
                                            in0=pm_ap, scalar1=b_ap,
                                            scalar2=None, op0=AL.add)
                    nc.vector.tensor_scalar(out=out_tile[:rows, :PR_PAD],
                                            in0=tmp_tile[:rows, :PR_PAD],
                                            scalar1=0.0, scalar2=None,
                                            op0=AL.max)
                    nc.vector.tensor_scalar(out=tmp_tile[:rows, :PR_PAD],
                                            in0=tmp_tile[:rows, :PR_PAD],
                                            scalar1=0.0, scalar2=RRELU_SLOPE,
                                            op0=AL.min, op1=AL.mult)
                    nc.vector.tensor_add(out=out_tile[:rows, :PR_PAD],
                                         in0=out_tile[:rows, :PR_PAD],
                                         in1=tmp_tile[:rows, :PR_PAD])

                pm1 = psf.tile([HID, CHW], F32, tag="f", space="PSUM",
                               name="pml1")
                nc.tensor.matmul(out=pm1[:, :PR_PAD], lhsT=l1w[:],
                                 rhs=pooled[:, :PR_PAD],
                                 start=True, stop=True)
                g1 = hp2.tile([HID, PR_PAD], F32, tag="pool", name="g1")
                t1 = hp2.tile([HID, PR_PAD], F32, tag="pool", name="t1")
                rrelu_block(pm1[:, :PR_PAD], l1b[:], HID, g1, t1)
                pm2 = psf.tile([HID, CHW], F32, tag="f", space="PSUM",
                               name="pml2")
                nc.tensor.matmul(out=pm2[:1, :PR_PAD], lhsT=l2w[:],
                                 rhs=g1[:, :PR_PAD],
                                 start=True, stop=True)
                g2 = hp2.tile([HID, PR_PAD], F32, tag="pool", name="g2")
                t2 = hp2.tile([HID, PR_PAD], F32, tag="pool", name="t2")
                rrelu_block(pm2[:1, :PR_PAD], l2b[:], 1, g2, t2)
                nc.sync.dma_start(out_t[:], g2[:1, :PR_PAD])

            def prologue():
                nc.sync.dma_start(tab[:], tab1_in[:])
                nc.vector.memset(slab[:, NPC:], 0.0)
                nc.vector.memset(pool_raw[:, GPC:], 0.0)

            if mode == "full":
                for _ in range(reps):
                    prologue()
                    for l in range(1, 6):
                        layer(l)
                        allgather()
                    nc.vector.memset(tab[:, NPC:], 0.0)
                    pooling_head()
            elif mode == "gather":
                prologue()
                for _ in range(reps):
                    for _l in range(5):
                        gather_reduce(echunks, gidx, tab, TBLW, slab)
                pooling_head()
            elif mode == "epi":
                prologue()
                gather_reduce(echunks, gidx, tab, TBLW, slab)
                for _ in range(reps):
                    for l in range(1, 6):
                        layer(l, do_gather=False)
                pooling_head()
            elif mode == "ag":
                prologue()
                nc.sync.dma_start(
                    bass.AP(ag_in[:].tensor, 0, [[D_PAD, HID], [1, D_PAD]]),
                    tab[:HID, :])
                for _ in range(reps):
                    for _l in range(5):
                        allgather()
                pooling_head()
            else:
                raise ValueError(mode)

    nc.finalize()
    return nc


# ---------------------------------------------------------------------------
# entry
# ---------------------------------------------------------------------------

def _make_in_maps(plan, per_core, inputs):
    tab1 = _make_tab1(inputs["x"], inputs["W1"], plan["dis"])
    repI = np.tile(np.eye(16, dtype=np.float32), (8, 1))
    repW = np.concatenate(
        [np.tile(np.asarray(inputs[f"W{i}"], np.float32), (8, 1))
         for i in range(2, 6)], axis=1)  # [128, 64]
    bs = np.tile(np.stack([np.asarray(inputs[f"b{i}"], np.float32)
                        for i in range(1, 6)], axis=1), (8, 1))
    in_maps = []
    for c in range(C):
        pc = per_core[c]
        in_maps.append({
            "tab1": tab1,
            "gidx": pc["gidx"], "aidx": pc["aidx"],
            "pidx": pc["pidx"], "paidx": pc["paidx"],
            "disT": pc["disT"], "repI": repI, "repW": repW,
            "rcpcnt": pc["rcpcnt"], "bs": bs,
            "lin1_w": np.asarray(inputs["lin1_w"], np.float32),
            "lin1_b": np.asarray(inputs["lin1_b"], np.float32).reshape(HID, 1),
            "lin2_w": np.asarray(inputs["lin2_w"], np.float32),
            "lin2_b": np.asarray(inputs["lin2_b"], np.float32).reshape(1, 1),
        })
    return in_maps


_CACHE = {}


def kernel(x, edge_index, batch, W1, b1, W2, b2, W3, b3, W4, b4, W5, b5,
           lin1_w, lin1_b, lin2_w, lin2_b, _reps=1, _prebuilt=None,
           _mode="full"):
    inputs = dict(x=x, edge_index=edge_index, batch=batch, W1=W1, b1=b1,
                  W2=W2, b2=b2, W3=W3, b3=b3, W4=W4, b4=b4, W5=W5, b5=b5,
                  lin1_w=lin1_w, lin1_b=lin1_b, lin2_w=lin2_w, lin2_b=lin2_b)
    import hashlib
    key = hashlib.sha1(
        np.ascontiguousarray(np.asarray(edge_index, np.int64)).tobytes()
        + np.ascontiguousarray(np.asarray(batch, np.int64)).tobytes()
    ).hexdigest() + f'|{_reps}|{_mode}'
    if key in _CACHE:
        plan, per_core, nc_cached = _CACHE[key]
    else:
        plan, per_core = _preprocess(x, edge_index, batch)
        nc_cached = _build_program(plan, reps=_reps, mode=_mode) \
            if _prebuilt is None else None
        _CACHE[key] = (plan, per_core, nc_cached)
    if _prebuilt is not None:
        nc = _prebuilt
    else:
        if nc_cached is None:
            nc_cached = _build_program(plan, reps=_reps, mode=_mode)
            _CACHE[key] = (plan, per_core, nc_cached)
        nc = nc_cached
    in_maps = _make_in_maps(plan, per_core, inputs)
    res = run_bass_via_pjrt(nc, in_maps, n_cores=C)
    out = np.zeros((N_GRAPHS, 1), dtype=np.float32)
    for c in range(C):
        out[c * GPC:(c + 1) * GPC, 0] = res[c]["out"][0, :GPC]
    return out
